# revision 23
# baseline (speedup 1.0000x reference)
"""MultiHeadCrossAttention Trainium2 kernel (8-core SPMD, query-parallel).

Sharding: core c handles batch b=c//4, query rows [1024*(c%4), +1024), all 8
heads.  Each core returns a disjoint [256, 1024] slice of out^T for its batch;
the host gather is a pure concat + transpose.

On-device layout is fully transposed ([channel, position]), matching the raw
[B, C, H, W] input layout, so no transposes are needed anywhere:
  q^T/k^T : [d, pos]   via  lhsT=W^T chunk [c,32|128], rhs=x^T chunk [c, pos]
  scores^T: [kpos, q]  via  lhsT=k^T [32,128] row-tiled 4x, rhs=q^T [32,512]
  exp     : ACT, PSUM->SBUF bf16, FD=1024 (the kernel's critical path:
            256 ACTIVATEs x ~1.0us is the ~260us floor)
  attn@v  : lhsT=[v|1] [128,33], rhs=p^T [128,512], col-tiled 2x (out
            partitions 0-32 / 64-96); the ones column yields softmax
            denominators in rows 32/96 of the accumulating matmuls
  norm    : denominators DMA-gathered into [128,64] tiles, approx-reciprocal,
            DMA back to a [1,NQ] row, gpsimd partition-broadcast, DVE mul
  final   : y^T accumulated in SBUF, one K=32 matmul + DVE add per head

Emission order software-pipelines head h+1's projections under head h's
attention so the ACT engine starts exp'ing within ~10us of kernel start.
All DVE/ACT ops keep in/out on identical partition ranges (walrus verifier
requirement); every cross-partition move rides on DMA or the PE.
"""

import numpy as np
import ml_dtypes

B, C, N, HEADS, D = 2, 256, 4096, 8, 32
NQ = 1024          # queries per core
NCORES = 8
CC = C // 128      # contraction chunks (2)

BF16 = ml_dtypes.bfloat16

_cached = {}
CFG = {"colattn": False, "cbase": 0, "scores4": True, "interleave": True, "norm": True, "debug": False,
       "dve_exp": True, "dve_rounds": (1, 3, 5)}


def _build_nc():
    import concourse.bass as bass
    import concourse.bacc as bacc
    import concourse.tile as tile
    import concourse.mybir as mybir
    from contextlib import ExitStack

    fp32 = mybir.dt.float32
    bf16 = mybir.dt.bfloat16
    i16 = mybir.dt.int16
    Exp = mybir.ActivationFunctionType.Exp

    nc = bacc.Bacc("TRN2", target_bir_lowering=False, debug=False,
                   num_devices=NCORES)

    src_d = nc.dram_tensor("src_bf", [C, N], bf16, kind="ExternalInput")
    tgt_d = nc.dram_tensor("tgt_bf", [C, NQ], bf16, kind="ExternalInput")
    tgt8_d = nc.dram_tensor("tgt8", [32, HEADS * NQ], bf16,
                            kind="ExternalInput")
    wq4_d = nc.dram_tensor("wq4", [C, HEADS * 128], bf16, kind="ExternalInput")
    wk_d = nc.dram_tensor("wkT", [C, C], bf16, kind="ExternalInput")
    wv_d = nc.dram_tensor("wvT", [C, C], bf16, kind="ExternalInput")
    wo8_d = nc.dram_tensor("wo8", [32, HEADS * C], bf16, kind="ExternalInput")
    y_d = nc.dram_tensor("yT", [C, NQ], fp32, kind="ExternalOutput")
    if CFG["debug"]:
        dbg_xw0_d = nc.dram_tensor("dbg_xw0", [32, NQ], fp32,
                                   kind="ExternalOutput")
        dbg_rbs0_d = nc.dram_tensor("dbg_rbs0", [32, NQ], fp32,
                                    kind="ExternalOutput")
        dbg_xf0_d = nc.dram_tensor("dbg_xf0", [32, NQ], fp32,
                                   kind="ExternalOutput")

    with tile.TileContext(nc) as tc, ExitStack() as ctx:
        konst = ctx.enter_context(tc.tile_pool(name="konst", bufs=1))
        work = ctx.enter_context(tc.tile_pool(name="work", bufs=1))
        p_pool = ctx.enter_context(tc.tile_pool(name="p", bufs=4))
        sm_pool = ctx.enter_context(tc.tile_pool(name="sm", bufs=2))
        xb_pool = ctx.enter_context(tc.tile_pool(name="xb", bufs=2))
        # PSUM budget (8 banks): ps tiles are [128,1024] (2 banks each),
        # po is [128,1024] when col-tiled else [128,512]
        po_bufs = 2 if CFG["colattn"] else 2
        ps_bufs = 2 if CFG["colattn"] else 3
        ps_pool = ctx.enter_context(tc.tile_pool(name="ps", bufs=ps_bufs, space="PSUM"))
        po_pool = ctx.enter_context(tc.tile_pool(name="po", bufs=po_bufs, space="PSUM"))
        pj_pool = po_pool

        # ---- load inputs (ordered so kproj/vproj deps land first; the DMA
        # stream is HBM-bound ~15us and overlaps the first attention rounds)
        src_sb = konst.tile([128, CC * N], bf16, tag="src")
        tgt_sb = konst.tile([128, CC * NQ], bf16, tag="tgt")
        tgt8_sb = konst.tile([32, HEADS * NQ], bf16, tag="tgt8")
        wq4_sb = konst.tile([128, CC * HEADS * 128], bf16, tag="wq4")
        wk_sb = konst.tile([128, CC * C], bf16, tag="wk")
        wv_sb = konst.tile([128, CC * C], bf16, tag="wv")
        wo8_sb = konst.tile([32, HEADS * C], bf16, tag="wo8")
        # copy of wo8 at partitions 64-96: lhsT for the chain-B recombine
        # matmul (compiler requires fmap/weight at the same partition base)
        wo8B_sb = (konst.tile([96, HEADS * C], bf16, tag="wo8B", name="wo8B")
                   if CFG["colattn"] else None)

        def dma_w(w_sb, w_d, eng=None):
            for cc in range(CC):
                (eng or nc.sync).dma_start(w_sb[:, cc * C:(cc + 1) * C],
                                           w_d.ap()[128 * cc:128 * (cc + 1), :])

        def dma_src_half(half, eng=None):
            for cc in range(CC):
                (eng or nc.sync).dma_start(
                    src_sb[:, cc * N + 2048 * half: cc * N + 2048 * (half + 1)],
                    src_d.ap()[128 * cc:128 * (cc + 1),
                               2048 * half:2048 * (half + 1)])

        # ordered so the first exp's exact deps land first: k_step(0) needs
        # wk + src half 0; q_step(0) needs only tgt cols 0-512 and wq4's
        # head-0 slice (1.5 MB total before the first scores round).  The
        # rest — q_step(1)/k_step(1) deps, wv — streams under the first
        # attention rounds (both steps are fed at rounds 3-4).
        dma_w(wk_sb, wk_d)
        dma_w(wv_sb, wv_d)
        for cc in range(CC):
            nc.sync.dma_start(tgt_sb[:, cc * NQ:cc * NQ + 512],
                              tgt_d.ap()[128 * cc:128 * (cc + 1), 0:512])
        for cc in range(CC):
            nc.sync.dma_start(wq4_sb[:, cc * 1024:cc * 1024 + 128],
                              wq4_d.ap()[128 * cc:128 * (cc + 1), 0:128])
        dma_src_half(0)
        for cc in range(CC):
            nc.sync.dma_start(tgt_sb[:, cc * NQ + 512:(cc + 1) * NQ],
                              tgt_d.ap()[128 * cc:128 * (cc + 1), 512:1024])
        dma_src_half(1)
        for cc in range(CC):
            nc.sync.dma_start(wq4_sb[:, cc * 1024 + 128:(cc + 1) * 1024],
                              wq4_d.ap()[128 * cc:128 * (cc + 1), 128:1024])
        nc.sync.dma_start(tgt8_sb[:], tgt8_d.ap()[:, :])
        nc.sync.dma_start(wo8_sb[:], wo8_d.ap()[:, :])
        if CFG["colattn"]:
            nc.sync.dma_start(wo8B_sb[64:96, :], wo8_d.ap()[:, :])

        # ---- persistent tiles ---------------------------------------------
        kT = [konst.tile([128, 1024], bf16, tag=f"kT{h}", name=f"kT{h}")
              for h in range(HEADS)]
        qT = [konst.tile([128, NQ], bf16, tag=f"qT{h}", name=f"qT{h}")
              for h in range(HEADS)]
        v_sb = konst.tile([128, HEADS * 33 * 32], bf16, tag="v")
        for h in range(HEADS):
            ones_ap = v_sb[:].rearrange("p (h k c) -> p h k c", h=HEADS, k=32)[
                :, h, :, 32:33]
            nc.gpsimd.memset(ones_ap, 1.0)
        xwh = [work.tile([33, NQ], fp32, tag=f"xw{h}", name=f"xw{h}")
               for h in range(HEADS)]
        # chain-B weighted sums live at partitions 64-96 (col-tiled attn@v);
        # they are recombined with chain A on the PE inside norm_mm
        xwB = [work.tile([96, NQ], bf16, tag=f"xwB{h}", name=f"xwB{h}")
               for h in range(HEADS)] if CFG["colattn"] else None
        # softmax denominators, one [16,64] tile per HEAD at partition base 0
        # (custom-DVE ops corrupt at base!=0 on HW): tile[8*qb+r, f] holds
        # queries 512*qb + 64*r .. +64.  Per-head tiles let recip(h) run
        # right after head h finishes, so normalize(h-1) hides under head h
        # instead of stacking up at the kernel tail.
        su = [work.tile([16, 64], fp32, tag=f"su{h}", name=f"su{h}")
              for h in range(HEADS)]
        suB = [work.tile([16, 64], fp32, tag=f"suB{h}", name=f"suB{h}")
               for h in range(HEADS)] if CFG["colattn"] else None
        rsum = [work.tile([16, 64], fp32, tag=f"rs{h}", name=f"rs{h}")
                for h in range(HEADS)]
        yacc = [work.tile([128, NQ], fp32, tag=f"yacc{t}", name=f"yacc{t}")
                for t in range(CC)]

        v_done = set()

        def vproj(kc):
            if kc in v_done:
                return
            v_done.add(kc)
            ps = pj_pool.tile([128, 512], fp32, tag="po", name=f"psv{kc}")
            for cc in range(CC):
                nc.tensor.matmul(
                    ps[:, 0:256],
                    lhsT=src_sb[:, cc * N + 128 * kc: cc * N + 128 * kc + 128],
                    rhs=wv_sb[:, cc * C:(cc + 1) * C],
                    start=(cc == 0), stop=(cc == CC - 1),
                    tile_position=(0, 0))
            dest = v_sb[:].rearrange("p (h k c) -> p h k c", h=HEADS, k=32)[
                :, :, kc, 0:32]
            nc.vector.tensor_copy(dest, ps[:, 0:256])

        def kqproj_steps(h):
            # k^T folded: strip g (partitions 32g..) holds kpos block b=4jj+g
            # at cols [512jj, +512); kc for 128-col slice m: 16*(m//4)+4g+(m%4)
            # Returned as small closures so callers can interleave them under
            # attention rounds (a single burst stalls ACT at head boundaries).
            steps = []
            state = {}

            def k_step(jj, gh):
                def run():
                    # half-step: strips 2*gh, 2*gh+1 only (smooths the PE
                    # burst across two feed slots)
                    ps = state.setdefault(jj, None)
                    if ps is None:
                        ps = pj_pool.tile([128, 512], fp32, tag="po",
                                          name=f"psk{h}_{jj}")
                        state[jj] = ps
                    for g in (2 * gh, 2 * gh + 1):
                        for cc in range(CC):
                            blk = 4 * jj + g
                            nc.tensor.matmul(
                                ps[32 * g:32 * g + 32, 0:512],
                                lhsT=wk_sb[:, cc * C + 32 * h: cc * C + 32 * h + 32],
                                rhs=src_sb[:, cc * N + 512 * blk: cc * N + 512 * blk + 512],
                                start=(cc == 0), stop=(cc == CC - 1),
                                tile_position=(0, 32 * g))
                    if gh == 1:
                        nc.vector.tensor_copy(
                            kT[h][:, 512 * jj:512 * jj + 512], ps[:, 0:512])
                        state[jj] = None
                return run

            def q_step(qb):
                def run():
                    ps = pj_pool.tile([128, 512], fp32, tag="po",
                                      name=f"psq{h}_{qb}")
                    for cc in range(CC):
                        nc.tensor.matmul(
                            ps[:, 0:512],
                            lhsT=wq4_sb[:, cc * 1024 + 128 * h: cc * 1024 + 128 * h + 128],
                            rhs=tgt_sb[:, cc * NQ + 512 * qb: cc * NQ + 512 * qb + 512],
                            start=(cc == 0), stop=(cc == CC - 1),
                            tile_position=(0, 0))
                    nc.vector.tensor_copy(qT[h][:, 512 * qb:512 * qb + 512],
                                          ps[:, 0:512])
                return run

            for jj in range(2):
                for gh in range(2):
                    steps.append(k_step(jj, gh))
            for qb in range(NQ // 512):
                steps.append(q_step(qb))
            return steps

        def kqproj(h):
            for st in kqproj_steps(h):
                st()

        def attn_unit(h, qb, feed=()):
            feed = list(feed)
            """One (head, 512-query-block) attention unit: 8 superrounds of
            4 k-chunks; scores row-tiled 4x across partition strips, attn@v
            col-tiled 2x (strips 0/1 -> out partitions 0-32, strips 2/3 ->
            64-96)."""
            po = po_pool.tile([128, 1024 if CFG["colattn"] else 512], fp32,
                              tag="po", name=f"po{h}_{qb}")
            rounds = []
            if CFG["scores4"]:
                rounds = [((0, 1, 2, 3), m) for m in range(8)]
            else:
                rounds = [((0, 1) if r % 2 == 0 else (2, 3), r // 2)
                          for r in range(16)]
            first = True
            for ri, (strips, m) in enumerate(rounds):
                last = ri == len(rounds) - 1
                if h == 0 and qb == 0:
                    for g in strips:
                        vproj(16 * (m // 4) + 4 * g + (m % 4))
                if feed:
                    feed.pop(0)()
                ntile = len(strips) // 2
                pss = [ps_pool.tile([128, 1024], fp32, tag="ps",
                                    name=f"ps{h}_{qb}_{ri}_{i}")
                       for i in range(ntile)]
                for gi, g in enumerate(strips):
                    nc.tensor.matmul(
                        pss[gi // 2][:, 512 * (gi % 2):512 * (gi % 2) + 512],
                        lhsT=kT[h][32 * g:32 * g + 32, 128 * m:128 * m + 128],
                        rhs=qT[h][32 * g:32 * g + 32, 512 * qb:512 * qb + 512],
                        start=True, stop=True,
                        tile_position=(32 * g, 0))
                pbs = []
                for i in range(ntile):
                    p_sb = p_pool.tile([128, 1024], bf16, tag="p",
                                       name=f"p{h}_{qb}_{ri}_{i}")
                    if (CFG["dve_exp"] and i == 1
                            and ri in CFG["dve_rounds"]):
                        # Schraudolph fast-exp on DVE: i16 = s*128*log2e + B,
                        # bitcast to bf16.  exp bias cancels in softmax; the
                        # +-2% mantissa-interp ripple is diluted ~40x by the
                        # residual (measured ~5e-4 on the final output).
                        nc.vector.tensor_scalar(
                            p_sb[:].bitcast(i16), pss[i][:, 0:1024],
                            184.6650558, 16250.0,
                            mybir.AluOpType.mult, mybir.AluOpType.add)
                    else:
                        nc.scalar.activation(p_sb[:], pss[i][:, 0:1024], Exp)
                    pbs.append(p_sb)
                for gi, g in enumerate(strips):
                    kc = 16 * (m // 4) + 4 * g + (m % 4)
                    if CFG["colattn"]:
                        co = CFG["cbase"] * (gi % 2)
                        fo = 512 * (gi % 2)
                        st = first and gi < 2
                        sp = last and gi >= len(strips) - 2
                    else:
                        co, fo = 0, 0
                        st = first and gi == 0
                        sp = last and gi == len(strips) - 1
                    nc.tensor.matmul(
                        po[co:co + 33, fo:fo + 512],
                        lhsT=v_sb[:, 1056 * h + 33 * kc: 1056 * h + 33 * kc + 33],
                        rhs=pbs[gi // 2][:, 512 * (gi % 2):512 * (gi % 2) + 512],
                        start=st, stop=sp,
                        tile_position=(0, co))
                first = False
            for st in feed:
                st()
            # drain weighted-sum rows AND denominator row 32 in one copy;
            # su gathers straight from xwh row 32
            nc.vector.tensor_copy(xwh[h][0:33, 512 * qb:512 * qb + 512],
                                  po[0:33, 0:512])
            nc.sync.dma_start(su[h][8 * qb:8 * qb + 8, 0:64],
                              xwh[h][32:33, 512 * qb:512 * qb + 512])
            if CFG["colattn"]:
                # chain B: data rows cb..cb+32, denominator row cb+32
                cb = CFG["cbase"]
                stmpB = sm_pool.tile([97, 512], fp32, tag="stmpB",
                                     name=f"stB{h}{qb}")
                nc.vector.tensor_copy(xwB[h][cb:cb + 32, 512 * qb:512 * qb + 512],
                                      po[cb:cb + 32, 512:1024])
                nc.vector.tensor_copy(stmpB[cb + 32:cb + 33, 0:512],
                                      po[cb + 32:cb + 33, 512:1024])
                nc.sync.dma_start(suB[h][8 * qb:8 * qb + 8, 0:64],
                                  stmpB[cb + 32:cb + 33, 0:512])

        def recip(h):
            if CFG["colattn"]:
                nc.vector.tensor_add(su[h][:], su[h][:], suB[h][:])
            nc.vector.reciprocal_approx_fast(rsum[h][:], su[h][:])

        xfh_t = {}

        def norm_dve(h):
            """Broadcast 1/denominator, scale w^T, add residual.  Emitted at
            the END of head h+1's drain: only gpsimd/DMA/DVE ops, so the
            broadcast latency never stalls the PE queue."""
            rrow = sm_pool.tile([1, NQ], fp32, tag="rrow", name=f"rr{h}")
            nc.sync.dma_start(rrow[:], rsum[h][:])
            nparts = 96 if (CFG["colattn"] and CFG["cbase"]) else 32
            rbs = sm_pool.tile([nparts, NQ], fp32, tag="rbs", name=f"rb{h}")
            nc.gpsimd.partition_broadcast(rbs[:], rrow[:])
            nc.vector.tensor_mul(xwh[h][0:32, :], xwh[h][0:32, :],
                                 rbs[0:32, :])
            xfh = xb_pool.tile([32, NQ], bf16, tag="xfh", name=f"xf{h}")
            nc.vector.tensor_add(xfh[:], xwh[h][0:32, :],
                                 tgt8_sb[:, NQ * h:NQ * (h + 1)])
            if CFG["colattn"]:
                cb = CFG["cbase"]
                xfB = xb_pool.tile([96, NQ], bf16, tag="xfB", name=f"xfB{h}")
                nc.vector.tensor_mul(xfB[cb:cb + 32, :], xwB[h][cb:cb + 32, :],
                                     rbs[cb:cb + 32, :])
                xfh_t[h] = (xfh, xfB)
            else:
                xfh_t[h] = xfh

        def norm_mm(h):
            """Project head h's normalized output and accumulate into y^T.
            Emitted mid-head h+2, when xfh (from norm_dve a full head earlier)
            is long ready — the PE never waits on the DVE chain."""
            xfh = xfh_t.pop(h)
            xfB = None
            if CFG["colattn"]:
                xfh, xfB = xfh
            for dc in range(CC):
                for qb in range(NQ // 512):
                    ps = pj_pool.tile([128, 512], fp32, tag="po",
                                      name=f"py{h}_{dc}_{qb}")
                    nc.tensor.matmul(
                        ps[:, 0:512],
                        lhsT=wo8_sb[:, C * h + 128 * dc: C * h + 128 * dc + 128],
                        rhs=xfh[:, 512 * qb:512 * qb + 512],
                        start=True, stop=(xfB is None), tile_position=(0, 0))
                    if xfB is not None:
                        cb = CFG["cbase"]
                        wB = (wo8B_sb[64:96] if cb else wo8_sb[0:32])
                        nc.tensor.matmul(
                            ps[:, 0:512],
                            lhsT=wB[:, C * h + 128 * dc: C * h + 128 * dc + 128],
                            rhs=xfB[cb:cb + 32, 512 * qb:512 * qb + 512],
                            start=False, stop=True, tile_position=(cb, 0))
                    if h == 0:
                        nc.vector.tensor_copy(
                            yacc[dc][:, 512 * qb:512 * qb + 512], ps[:, 0:512])
                    else:
                        nc.vector.tensor_add(
                            yacc[dc][:, 512 * qb:512 * qb + 512],
                            yacc[dc][:, 512 * qb:512 * qb + 512], ps[:, 0:512])
                if h == HEADS - 1:
                    # overlap the output store per 128-row block
                    nc.sync.dma_start(y_d.ap()[128 * dc:128 * (dc + 1), :],
                                      yacc[dc][:])

        # ---- emission: software-pipeline projections under attention ------
        if CFG["interleave"]:
            # head-0 ramp: only k_step(0)+q emit up front (they need just
            # wk/tgt/wq4/src-half-0); k_step(1) feeds at round 2, by which
            # time src half 1 has streamed in.  Rounds 0-3 touch kT cols
            # 0-512 (jj=0) only.
            s0 = kqproj_steps(0)
            s0[0]()
            s0[1]()
            s0[4]()
            noop = lambda: None  # noqa: E731
            for h in range(HEADS):
                steps = kqproj_steps(h + 1) if h + 1 < HEADS else []
                if h == 0:
                    attn_unit(h, 0,
                              feed=[noop, noop, noop, s0[2], s0[3], s0[5]]
                              + steps[:2])
                    attn_unit(h, 1, feed=steps[2:])
                else:
                    attn_unit(h, 0, feed=steps)
                    if h >= 2:
                        norm_mm(h - 2)
                    if h == 7:
                        norm_mm(6)
                    attn_unit(h, 1)
                recip(h)
                if 1 <= h <= 5:
                    norm_dve(h - 1)
                if h == 6:
                    # lag-0 for head 6 only: its broadcast-blocked DVE burst
                    # hides fully (head 7 has no fed CASTs), and norm_mm(6)
                    # then hides under head 7 — the tail is just head 7's
                    # own normalize
                    norm_dve(5)
                    norm_dve(6)
            norm_dve(HEADS - 1)
            norm_mm(HEADS - 1)

    nc.compile()
    return nc


def _prep_core_inputs(core, tgt, src, Wq, Wk, Wv, Wo):
    b, qoff = core // 4, NQ * (core % 4)
    srcT = src[b].reshape(C, N)
    tgtT = tgt[b].reshape(C, N)[:, qoff:qoff + NQ]
    scale = 1.0 / np.sqrt(np.float32(D))
    wqT = (Wq * scale).T.astype(BF16)
    wq4 = np.empty((C, HEADS * 128), dtype=BF16)
    for h in range(HEADS):
        wq4[:, 128 * h:128 * (h + 1)] = np.tile(wqT[:, 32 * h:32 * h + 32],
                                                (1, 4))
    # per-head row blocks of tgt^T / Wo^T laid side by side at partitions 0-31
    tgt8 = np.empty((32, HEADS * NQ), dtype=BF16)
    woT = Wo.T.astype(np.float32)
    wo8 = np.empty((32, HEADS * C), dtype=BF16)
    for h in range(HEADS):
        tgt8[:, NQ * h:NQ * (h + 1)] = tgtT[32 * h:32 * h + 32, :]
        wo8[:, C * h:C * (h + 1)] = woT[32 * h:32 * h + 32, :].astype(BF16)
    return {
        "src_bf": np.ascontiguousarray(srcT).astype(BF16),
        "tgt_bf": np.ascontiguousarray(tgtT).astype(BF16),
        "tgt8": tgt8,
        "wq4": wq4,
        "wkT": np.ascontiguousarray(Wk.T).astype(BF16),
        "wvT": np.ascontiguousarray(Wv.T).astype(BF16),
        "wo8": wo8,
    }


def kernel(tgt, src, Wq, Wk, Wv, Wo, _want_results=False):
    from concourse.bass_utils import run_bass_kernel_spmd

    tgt = np.asarray(tgt, dtype=np.float32)
    src = np.asarray(src, dtype=np.float32)
    Wq = np.asarray(Wq, dtype=np.float32)
    Wk = np.asarray(Wk, dtype=np.float32)
    Wv = np.asarray(Wv, dtype=np.float32)
    Wo = np.asarray(Wo, dtype=np.float32)

    if "nc" not in _cached:
        _cached["nc"] = _build_nc()
    nc = _cached["nc"]

    in_maps = [_prep_core_inputs(c, tgt, src, Wq, Wk, Wv, Wo)
               for c in range(NCORES)]
    res = run_bass_kernel_spmd(nc, in_maps, core_ids=list(range(NCORES)))

    out = np.empty((B, N, C), dtype=np.float32)
    for c in range(NCORES):
        b, qoff = c // 4, NQ * (c % 4)
        out[b, qoff:qoff + NQ, :] = res.results[c]["yT"].T
    if _want_results:
        return out, res
    return out



# revision 25
# speedup vs baseline: 1.0117x; 1.0117x over previous
"""MultiHeadCrossAttention Trainium2 kernel (8-core SPMD, query-parallel).

Sharding: core c handles batch b=c//4, query rows [1024*(c%4), +1024), all 8
heads.  Each core returns a disjoint [256, 1024] slice of out^T for its batch;
the host gather is a pure concat + transpose.

On-device layout is fully transposed ([channel, position]), matching the raw
[B, C, H, W] input layout, so no transposes are needed anywhere:
  q^T/k^T : [d, pos]   via  lhsT=W^T chunk [c,32|128], rhs=x^T chunk [c, pos]
  scores^T: [kpos, q]  via  lhsT=k^T [32,128] row-tiled 4x, rhs=q^T [32,512]
  exp     : ACT, PSUM->SBUF bf16, FD=1024 (the kernel's critical path:
            256 ACTIVATEs x ~1.0us is the ~260us floor)
  attn@v  : lhsT=[v|1] [128,33], rhs=p^T [128,512], col-tiled 2x (out
            partitions 0-32 / 64-96); the ones column yields softmax
            denominators in rows 32/96 of the accumulating matmuls
  norm    : denominators DMA-gathered into [128,64] tiles, approx-reciprocal,
            DMA back to a [1,NQ] row, gpsimd partition-broadcast, DVE mul
  final   : y^T accumulated in SBUF, one K=32 matmul + DVE add per head

Emission order software-pipelines head h+1's projections under head h's
attention so the ACT engine starts exp'ing within ~10us of kernel start.
All DVE/ACT ops keep in/out on identical partition ranges (walrus verifier
requirement); every cross-partition move rides on DMA or the PE.
"""

import numpy as np
import ml_dtypes

B, C, N, HEADS, D = 2, 256, 4096, 8, 32
NQ = 1024          # queries per core
NCORES = 8
CC = C // 128      # contraction chunks (2)

BF16 = ml_dtypes.bfloat16

_cached = {}
CFG = {"colattn": False, "cbase": 0, "scores4": True, "interleave": True, "norm": True, "debug": False,
       "dve_exp": True, "dve_rounds": (1, 3, 5)}


def _build_nc():
    import concourse.bass as bass
    import concourse.bacc as bacc
    import concourse.tile as tile
    import concourse.mybir as mybir
    from contextlib import ExitStack

    fp32 = mybir.dt.float32
    bf16 = mybir.dt.bfloat16
    i16 = mybir.dt.int16
    Exp = mybir.ActivationFunctionType.Exp

    nc = bacc.Bacc("TRN2", target_bir_lowering=False, debug=False,
                   num_devices=NCORES)

    src_d = nc.dram_tensor("src_bf", [C, N], bf16, kind="ExternalInput")
    tgt_d = nc.dram_tensor("tgt_bf", [C, NQ], bf16, kind="ExternalInput")
    tgt8_d = nc.dram_tensor("tgt8", [32, HEADS * NQ], bf16,
                            kind="ExternalInput")
    wq4_d = nc.dram_tensor("wq4", [C, HEADS * 128], bf16, kind="ExternalInput")
    wk_d = nc.dram_tensor("wkT", [C, C], bf16, kind="ExternalInput")
    wv_d = nc.dram_tensor("wvT", [C, C], bf16, kind="ExternalInput")
    wo8_d = nc.dram_tensor("wo8", [32, HEADS * C], bf16, kind="ExternalInput")
    y_d = nc.dram_tensor("yT", [C, NQ], fp32, kind="ExternalOutput")
    if CFG["debug"]:
        dbg_xw0_d = nc.dram_tensor("dbg_xw0", [32, NQ], fp32,
                                   kind="ExternalOutput")
        dbg_rbs0_d = nc.dram_tensor("dbg_rbs0", [32, NQ], fp32,
                                    kind="ExternalOutput")
        dbg_xf0_d = nc.dram_tensor("dbg_xf0", [32, NQ], fp32,
                                   kind="ExternalOutput")

    with tile.TileContext(nc) as tc, ExitStack() as ctx:
        konst = ctx.enter_context(tc.tile_pool(name="konst", bufs=1))
        work = ctx.enter_context(tc.tile_pool(name="work", bufs=1))
        p_pool = ctx.enter_context(tc.tile_pool(name="p", bufs=4))
        sm_pool = ctx.enter_context(tc.tile_pool(name="sm", bufs=2))
        xb_pool = ctx.enter_context(tc.tile_pool(name="xb", bufs=2))
        # PSUM budget (8 banks): ps tiles are [128,1024] (2 banks each),
        # po is [128,1024] when col-tiled else [128,512]
        po_bufs = 2 if CFG["colattn"] else 2
        ps_bufs = 2 if CFG["colattn"] else 3
        ps_pool = ctx.enter_context(tc.tile_pool(name="ps", bufs=ps_bufs, space="PSUM"))
        po_pool = ctx.enter_context(tc.tile_pool(name="po", bufs=po_bufs, space="PSUM"))
        pj_pool = po_pool

        # ---- load inputs (ordered so kproj/vproj deps land first; the DMA
        # stream is HBM-bound ~15us and overlaps the first attention rounds)
        src_sb = konst.tile([128, CC * N], bf16, tag="src")
        tgt_sb = konst.tile([128, CC * NQ], bf16, tag="tgt")
        tgt8_sb = konst.tile([32, HEADS * NQ], bf16, tag="tgt8")
        wq4_sb = konst.tile([128, CC * HEADS * 128], bf16, tag="wq4")
        wk_sb = konst.tile([128, CC * C], bf16, tag="wk")
        wv_sb = konst.tile([128, CC * C], bf16, tag="wv")
        wo8_sb = konst.tile([32, HEADS * C], bf16, tag="wo8")
        # copy of wo8 at partitions 64-96: lhsT for the chain-B recombine
        # matmul (compiler requires fmap/weight at the same partition base)
        wo8B_sb = (konst.tile([96, HEADS * C], bf16, tag="wo8B", name="wo8B")
                   if CFG["colattn"] else None)

        def dma_w(w_sb, w_d, eng=None):
            for cc in range(CC):
                (eng or nc.sync).dma_start(w_sb[:, cc * C:(cc + 1) * C],
                                           w_d.ap()[128 * cc:128 * (cc + 1), :])

        def dma_src_half(half, eng=None):
            for cc in range(CC):
                (eng or nc.sync).dma_start(
                    src_sb[:, cc * N + 2048 * half: cc * N + 2048 * (half + 1)],
                    src_d.ap()[128 * cc:128 * (cc + 1),
                               2048 * half:2048 * (half + 1)])

        # ordered so the first exp's exact deps land first: k_step(0) needs
        # wk + src half 0; q_step(0) needs only tgt cols 0-512 and wq4's
        # head-0 slice (1.5 MB total before the first scores round).  The
        # rest — q_step(1)/k_step(1) deps, wv — streams under the first
        # attention rounds (both steps are fed at rounds 3-4).
        dma_w(wk_sb, wk_d)
        dma_w(wv_sb, wv_d)
        for cc in range(CC):
            nc.sync.dma_start(tgt_sb[:, cc * NQ:cc * NQ + 512],
                              tgt_d.ap()[128 * cc:128 * (cc + 1), 0:512])
        for cc in range(CC):
            nc.sync.dma_start(wq4_sb[:, cc * 1024:cc * 1024 + 128],
                              wq4_d.ap()[128 * cc:128 * (cc + 1), 0:128])
        # src half 0 in 1024-col chunks, cc-interleaved: k_step(0, gh=0)
        # (blocks 0-1) only needs the first two chunks (~0.5 MB)
        for quarter in range(2):
            for cc in range(CC):
                nc.sync.dma_start(
                    src_sb[:, cc * N + 1024 * quarter: cc * N + 1024 * (quarter + 1)],
                    src_d.ap()[128 * cc:128 * (cc + 1),
                               1024 * quarter:1024 * (quarter + 1)])
        for cc in range(CC):
            nc.sync.dma_start(tgt_sb[:, cc * NQ + 512:(cc + 1) * NQ],
                              tgt_d.ap()[128 * cc:128 * (cc + 1), 512:1024])
        dma_src_half(1)
        for cc in range(CC):
            nc.sync.dma_start(wq4_sb[:, cc * 1024 + 128:(cc + 1) * 1024],
                              wq4_d.ap()[128 * cc:128 * (cc + 1), 128:1024])
        nc.sync.dma_start(tgt8_sb[:], tgt8_d.ap()[:, :])
        nc.sync.dma_start(wo8_sb[:], wo8_d.ap()[:, :])
        if CFG["colattn"]:
            nc.sync.dma_start(wo8B_sb[64:96, :], wo8_d.ap()[:, :])

        # ---- persistent tiles ---------------------------------------------
        kT = [konst.tile([128, 1024], bf16, tag=f"kT{h}", name=f"kT{h}")
              for h in range(HEADS)]
        qT = [konst.tile([128, NQ], bf16, tag=f"qT{h}", name=f"qT{h}")
              for h in range(HEADS)]
        v_sb = konst.tile([128, HEADS * 33 * 32], bf16, tag="v")
        for h in range(HEADS):
            ones_ap = v_sb[:].rearrange("p (h k c) -> p h k c", h=HEADS, k=32)[
                :, h, :, 32:33]
            nc.gpsimd.memset(ones_ap, 1.0)
        xwh = [work.tile([33, NQ], fp32, tag=f"xw{h}", name=f"xw{h}")
               for h in range(HEADS)]
        # chain-B weighted sums live at partitions 64-96 (col-tiled attn@v);
        # they are recombined with chain A on the PE inside norm_mm
        xwB = [work.tile([96, NQ], bf16, tag=f"xwB{h}", name=f"xwB{h}")
               for h in range(HEADS)] if CFG["colattn"] else None
        # softmax denominators, one [16,64] tile per HEAD at partition base 0
        # (custom-DVE ops corrupt at base!=0 on HW): tile[8*qb+r, f] holds
        # queries 512*qb + 64*r .. +64.  Per-head tiles let recip(h) run
        # right after head h finishes, so normalize(h-1) hides under head h
        # instead of stacking up at the kernel tail.
        su = [work.tile([16, 64], fp32, tag=f"su{h}", name=f"su{h}")
              for h in range(HEADS)]
        # head-7 denominators per qb at base 0 (custom-DVE recip needs base 0)
        su7q = [work.tile([8, 64], fp32, tag=f"su7q{qb}", name=f"su7q{qb}")
                for qb in range(2)]
        rs7q = [work.tile([8, 64], fp32, tag=f"rs7q{qb}", name=f"rs7q{qb}")
                for qb in range(2)]
        suB = [work.tile([16, 64], fp32, tag=f"suB{h}", name=f"suB{h}")
               for h in range(HEADS)] if CFG["colattn"] else None
        rsum = [work.tile([16, 64], fp32, tag=f"rs{h}", name=f"rs{h}")
                for h in range(HEADS)]
        yacc = [work.tile([128, NQ], fp32, tag=f"yacc{t}", name=f"yacc{t}")
                for t in range(CC)]

        v_done = set()

        def vproj(kc):
            if kc in v_done:
                return
            v_done.add(kc)
            ps = pj_pool.tile([128, 512], fp32, tag="po", name=f"psv{kc}")
            for cc in range(CC):
                nc.tensor.matmul(
                    ps[:, 0:256],
                    lhsT=src_sb[:, cc * N + 128 * kc: cc * N + 128 * kc + 128],
                    rhs=wv_sb[:, cc * C:(cc + 1) * C],
                    start=(cc == 0), stop=(cc == CC - 1),
                    tile_position=(0, 0))
            dest = v_sb[:].rearrange("p (h k c) -> p h k c", h=HEADS, k=32)[
                :, :, kc, 0:32]
            nc.vector.tensor_copy(dest, ps[:, 0:256])

        def kqproj_steps(h):
            # k^T folded: strip g (partitions 32g..) holds kpos block b=4jj+g
            # at cols [512jj, +512); kc for 128-col slice m: 16*(m//4)+4g+(m%4)
            # Returned as small closures so callers can interleave them under
            # attention rounds (a single burst stalls ACT at head boundaries).
            steps = []
            state = {}

            def k_step(jj, gh):
                def run():
                    # half-step: strips 2*gh, 2*gh+1 only (smooths the PE
                    # burst across two feed slots)
                    ps = state.setdefault(jj, None)
                    if ps is None:
                        ps = pj_pool.tile([128, 512], fp32, tag="po",
                                          name=f"psk{h}_{jj}")
                        state[jj] = ps
                    for g in (2 * gh, 2 * gh + 1):
                        for cc in range(CC):
                            blk = 4 * jj + g
                            nc.tensor.matmul(
                                ps[32 * g:32 * g + 32, 0:512],
                                lhsT=wk_sb[:, cc * C + 32 * h: cc * C + 32 * h + 32],
                                rhs=src_sb[:, cc * N + 512 * blk: cc * N + 512 * blk + 512],
                                start=(cc == 0), stop=(cc == CC - 1),
                                tile_position=(0, 32 * g))
                    if gh == 1:
                        nc.vector.tensor_copy(
                            kT[h][:, 512 * jj:512 * jj + 512], ps[:, 0:512])
                        state[jj] = None
                return run

            def q_step(qb):
                def run():
                    ps = pj_pool.tile([128, 512], fp32, tag="po",
                                      name=f"psq{h}_{qb}")
                    for cc in range(CC):
                        nc.tensor.matmul(
                            ps[:, 0:512],
                            lhsT=wq4_sb[:, cc * 1024 + 128 * h: cc * 1024 + 128 * h + 128],
                            rhs=tgt_sb[:, cc * NQ + 512 * qb: cc * NQ + 512 * qb + 512],
                            start=(cc == 0), stop=(cc == CC - 1),
                            tile_position=(0, 0))
                    nc.vector.tensor_copy(qT[h][:, 512 * qb:512 * qb + 512],
                                          ps[:, 0:512])
                return run

            for jj in range(2):
                for gh in range(2):
                    steps.append(k_step(jj, gh))
            for qb in range(NQ // 512):
                steps.append(q_step(qb))
            return steps

        def kqproj(h):
            for st in kqproj_steps(h):
                st()

        def attn_unit(h, qb, feed=()):
            feed = list(feed)
            """One (head, 512-query-block) attention unit: 8 superrounds of
            4 k-chunks; scores row-tiled 4x across partition strips, attn@v
            col-tiled 2x (strips 0/1 -> out partitions 0-32, strips 2/3 ->
            64-96)."""
            po = po_pool.tile([128, 1024 if CFG["colattn"] else 512], fp32,
                              tag="po", name=f"po{h}_{qb}")
            rounds = []
            if CFG["scores4"]:
                rounds = [((0, 1, 2, 3), m) for m in range(8)]
            else:
                rounds = [((0, 1) if r % 2 == 0 else (2, 3), r // 2)
                          for r in range(16)]
            first = True
            for ri, (strips, m) in enumerate(rounds):
                last = ri == len(rounds) - 1
                if h == 0 and qb == 0:
                    for g in strips:
                        vproj(16 * (m // 4) + 4 * g + (m % 4))
                if feed:
                    feed.pop(0)()
                ntile = len(strips) // 2
                pss = [ps_pool.tile([128, 1024], fp32, tag="ps",
                                    name=f"ps{h}_{qb}_{ri}_{i}")
                       for i in range(ntile)]
                for gi, g in enumerate(strips):
                    nc.tensor.matmul(
                        pss[gi // 2][:, 512 * (gi % 2):512 * (gi % 2) + 512],
                        lhsT=kT[h][32 * g:32 * g + 32, 128 * m:128 * m + 128],
                        rhs=qT[h][32 * g:32 * g + 32, 512 * qb:512 * qb + 512],
                        start=True, stop=True,
                        tile_position=(32 * g, 0))
                pbs = []
                for i in range(ntile):
                    p_sb = p_pool.tile([128, 1024], bf16, tag="p",
                                       name=f"p{h}_{qb}_{ri}_{i}")
                    if (CFG["dve_exp"] and i == 1
                            and ri in CFG["dve_rounds"]):
                        # Schraudolph fast-exp on DVE: i16 = s*128*log2e + B,
                        # bitcast to bf16.  exp bias cancels in softmax; the
                        # +-2% mantissa-interp ripple is diluted ~40x by the
                        # residual (measured ~5e-4 on the final output).
                        nc.vector.tensor_scalar(
                            p_sb[:].bitcast(i16), pss[i][:, 0:1024],
                            184.6650558, 16250.0,
                            mybir.AluOpType.mult, mybir.AluOpType.add)
                    else:
                        nc.scalar.activation(p_sb[:], pss[i][:, 0:1024], Exp)
                    pbs.append(p_sb)
                for gi, g in enumerate(strips):
                    kc = 16 * (m // 4) + 4 * g + (m % 4)
                    if CFG["colattn"]:
                        co = CFG["cbase"] * (gi % 2)
                        fo = 512 * (gi % 2)
                        st = first and gi < 2
                        sp = last and gi >= len(strips) - 2
                    else:
                        co, fo = 0, 0
                        st = first and gi == 0
                        sp = last and gi == len(strips) - 1
                    nc.tensor.matmul(
                        po[co:co + 33, fo:fo + 512],
                        lhsT=v_sb[:, 1056 * h + 33 * kc: 1056 * h + 33 * kc + 33],
                        rhs=pbs[gi // 2][:, 512 * (gi % 2):512 * (gi % 2) + 512],
                        start=st, stop=sp,
                        tile_position=(0, co))
                first = False
            for st in feed:
                st()
            # drain weighted-sum rows AND denominator row 32 in one copy;
            # su gathers straight from xwh row 32
            nc.vector.tensor_copy(xwh[h][0:33, 512 * qb:512 * qb + 512],
                                  po[0:33, 0:512])
            su_dst = (su7q[qb][0:8, 0:64] if h == HEADS - 1
                      else su[h][8 * qb:8 * qb + 8, 0:64])
            nc.sync.dma_start(su_dst,
                              xwh[h][32:33, 512 * qb:512 * qb + 512])
            if CFG["colattn"]:
                # chain B: data rows cb..cb+32, denominator row cb+32
                cb = CFG["cbase"]
                stmpB = sm_pool.tile([97, 512], fp32, tag="stmpB",
                                     name=f"stB{h}{qb}")
                nc.vector.tensor_copy(xwB[h][cb:cb + 32, 512 * qb:512 * qb + 512],
                                      po[cb:cb + 32, 512:1024])
                nc.vector.tensor_copy(stmpB[cb + 32:cb + 33, 0:512],
                                      po[cb + 32:cb + 33, 512:1024])
                nc.sync.dma_start(suB[h][8 * qb:8 * qb + 8, 0:64],
                                  stmpB[cb + 32:cb + 33, 0:512])

        def recip(h):
            if CFG["colattn"]:
                nc.vector.tensor_add(su[h][:], su[h][:], suB[h][:])
            nc.vector.reciprocal_approx_fast(rsum[h][:], su[h][:])

        xfh_t = {}
        xfh7 = [xb_pool.tile([32, 512], bf16, tag=f"xf7{qb}", name=f"xf7{qb}")
                for qb in range(2)]

        def norm_dve(h):
            """Broadcast 1/denominator, scale w^T, add residual.  Emitted at
            the END of head h+1's drain: only gpsimd/DMA/DVE ops, so the
            broadcast latency never stalls the PE queue."""
            rrow = sm_pool.tile([1, NQ], fp32, tag="rrow", name=f"rr{h}")
            nc.sync.dma_start(rrow[:], rsum[h][:])
            nparts = 96 if (CFG["colattn"] and CFG["cbase"]) else 32
            rbs = sm_pool.tile([nparts, NQ], fp32, tag="rbs", name=f"rb{h}")
            nc.gpsimd.partition_broadcast(rbs[:], rrow[:])
            nc.vector.tensor_mul(xwh[h][0:32, :], xwh[h][0:32, :],
                                 rbs[0:32, :])
            xfh = xb_pool.tile([32, NQ], bf16, tag="xfh", name=f"xf{h}")
            nc.vector.tensor_add(xfh[:], xwh[h][0:32, :],
                                 tgt8_sb[:, NQ * h:NQ * (h + 1)])
            if CFG["colattn"]:
                cb = CFG["cbase"]
                xfB = xb_pool.tile([96, NQ], bf16, tag="xfB", name=f"xfB{h}")
                nc.vector.tensor_mul(xfB[cb:cb + 32, :], xwB[h][cb:cb + 32, :],
                                     rbs[cb:cb + 32, :])
                xfh_t[h] = (xfh, xfB)
            else:
                xfh_t[h] = xfh

        rbs7 = {}

        def norm7_recip(qb):
            # head-7 qb-half: recip + broadcast only (no DVE ops that would
            # block the queue on the gpsimd broadcast)
            nc.vector.reciprocal_approx_fast(rs7q[qb][:], su7q[qb][:])
            rrow = sm_pool.tile([1, 512], fp32, tag="rr7", name=f"rr7{qb}")
            nc.sync.dma_start(rrow[:], rs7q[qb][:])
            rbs = sm_pool.tile([32, 512], fp32, tag="rb7", name=f"rb7{qb}")
            nc.gpsimd.partition_broadcast(rbs[:], rrow[:])
            rbs7[qb] = rbs

        def norm7_dve(qb):
            h = HEADS - 1
            nc.vector.tensor_mul(xwh[h][0:32, 512 * qb:512 * qb + 512],
                                 xwh[h][0:32, 512 * qb:512 * qb + 512],
                                 rbs7[qb][:])
            xfh = xfh7[qb]
            nc.vector.tensor_add(
                xfh[:], xwh[h][0:32, 512 * qb:512 * qb + 512],
                tgt8_sb[:, NQ * h + 512 * qb:NQ * h + 512 * qb + 512])

        def norm7_mm_steps(qb):
            h = HEADS - 1
            steps = []
            for dc in range(CC):
                def run(dc=dc):
                    ps = pj_pool.tile([128, 512], fp32, tag="po",
                                      name=f"py7_{dc}_{qb}")
                    nc.tensor.matmul(
                        ps[:, 0:512],
                        lhsT=wo8_sb[:, C * h + 128 * dc: C * h + 128 * dc + 128],
                        rhs=xfh7[qb][:],
                        start=True, stop=True, tile_position=(0, 0))
                    nc.vector.tensor_add(
                        yacc[dc][:, 512 * qb:512 * qb + 512],
                        yacc[dc][:, 512 * qb:512 * qb + 512], ps[:, 0:512])
                    if qb == 1:
                        nc.sync.dma_start(y_d.ap()[128 * dc:128 * (dc + 1), :],
                                          yacc[dc][:])
                steps.append(run)
            return steps

        def norm_mm(h):
            """Project head h's normalized output and accumulate into y^T.
            Emitted mid-head h+2, when xfh (from norm_dve a full head earlier)
            is long ready — the PE never waits on the DVE chain."""
            xfh = xfh_t.pop(h)
            xfB = None
            if CFG["colattn"]:
                xfh, xfB = xfh
            for dc in range(CC):
                for qb in range(NQ // 512):
                    ps = pj_pool.tile([128, 512], fp32, tag="po",
                                      name=f"py{h}_{dc}_{qb}")
                    nc.tensor.matmul(
                        ps[:, 0:512],
                        lhsT=wo8_sb[:, C * h + 128 * dc: C * h + 128 * dc + 128],
                        rhs=xfh[:, 512 * qb:512 * qb + 512],
                        start=True, stop=(xfB is None), tile_position=(0, 0))
                    if xfB is not None:
                        cb = CFG["cbase"]
                        wB = (wo8B_sb[64:96] if cb else wo8_sb[0:32])
                        nc.tensor.matmul(
                            ps[:, 0:512],
                            lhsT=wB[:, C * h + 128 * dc: C * h + 128 * dc + 128],
                            rhs=xfB[cb:cb + 32, 512 * qb:512 * qb + 512],
                            start=False, stop=True, tile_position=(cb, 0))
                    if h == 0:
                        nc.vector.tensor_copy(
                            yacc[dc][:, 512 * qb:512 * qb + 512], ps[:, 0:512])
                    else:
                        nc.vector.tensor_add(
                            yacc[dc][:, 512 * qb:512 * qb + 512],
                            yacc[dc][:, 512 * qb:512 * qb + 512], ps[:, 0:512])
                if h == HEADS - 1:
                    # overlap the output store per 128-row block
                    nc.sync.dma_start(y_d.ap()[128 * dc:128 * (dc + 1), :],
                                      yacc[dc][:])

        # ---- emission: software-pipeline projections under attention ------
        if CFG["interleave"]:
            # head-0 ramp: only k_step(0)+q emit up front (they need just
            # wk/tgt/wq4/src-half-0); k_step(1) feeds at round 2, by which
            # time src half 1 has streamed in.  Rounds 0-3 touch kT cols
            # 0-512 (jj=0) only.
            s0 = kqproj_steps(0)
            s0[0]()
            s0[1]()
            s0[4]()
            noop = lambda: None  # noqa: E731
            for h in range(HEADS):
                steps = kqproj_steps(h + 1) if h + 1 < HEADS else []
                if h == 0:
                    attn_unit(h, 0,
                              feed=[noop, noop, noop, s0[2], s0[3], s0[5]]
                              + steps[:2])
                    attn_unit(h, 1, feed=steps[2:])
                else:
                    attn_unit(h, 0, feed=steps)
                    if h >= 2:
                        norm_mm(h - 2)
                    if h == 7:
                        norm_mm(6)
                        norm7_recip(0)
                        attn_unit(h, 1,
                                  feed=[noop, noop, (lambda: norm7_dve(0)),
                                        noop] + norm7_mm_steps(0))
                    else:
                        attn_unit(h, 1)
                if h < 7:
                    recip(h)
                if 1 <= h <= 5:
                    norm_dve(h - 1)
                if h == 6:
                    # lag-0 for head 6 only: its broadcast-blocked DVE burst
                    # hides fully (head 7 has no fed CASTs), and norm_mm(6)
                    # then hides under head 7 — the tail is just head 7's
                    # own normalize
                    norm_dve(5)
                    norm_dve(6)
            norm7_recip(1)
            norm7_dve(1)
            for st in norm7_mm_steps(1):
                st()

    nc.compile()
    return nc


def _prep_core_inputs(core, tgt, src, Wq, Wk, Wv, Wo):
    b, qoff = core // 4, NQ * (core % 4)
    srcT = src[b].reshape(C, N)
    tgtT = tgt[b].reshape(C, N)[:, qoff:qoff + NQ]
    scale = 1.0 / np.sqrt(np.float32(D))
    wqT = (Wq * scale).T.astype(BF16)
    wq4 = np.empty((C, HEADS * 128), dtype=BF16)
    for h in range(HEADS):
        wq4[:, 128 * h:128 * (h + 1)] = np.tile(wqT[:, 32 * h:32 * h + 32],
                                                (1, 4))
    # per-head row blocks of tgt^T / Wo^T laid side by side at partitions 0-31
    tgt8 = np.empty((32, HEADS * NQ), dtype=BF16)
    woT = Wo.T.astype(np.float32)
    wo8 = np.empty((32, HEADS * C), dtype=BF16)
    for h in range(HEADS):
        tgt8[:, NQ * h:NQ * (h + 1)] = tgtT[32 * h:32 * h + 32, :]
        wo8[:, C * h:C * (h + 1)] = woT[32 * h:32 * h + 32, :].astype(BF16)
    return {
        "src_bf": np.ascontiguousarray(srcT).astype(BF16),
        "tgt_bf": np.ascontiguousarray(tgtT).astype(BF16),
        "tgt8": tgt8,
        "wq4": wq4,
        "wkT": np.ascontiguousarray(Wk.T).astype(BF16),
        "wvT": np.ascontiguousarray(Wv.T).astype(BF16),
        "wo8": wo8,
    }


def kernel(tgt, src, Wq, Wk, Wv, Wo, _want_results=False):
    from concourse.bass_utils import run_bass_kernel_spmd

    tgt = np.asarray(tgt, dtype=np.float32)
    src = np.asarray(src, dtype=np.float32)
    Wq = np.asarray(Wq, dtype=np.float32)
    Wk = np.asarray(Wk, dtype=np.float32)
    Wv = np.asarray(Wv, dtype=np.float32)
    Wo = np.asarray(Wo, dtype=np.float32)

    if "nc" not in _cached:
        _cached["nc"] = _build_nc()
    nc = _cached["nc"]

    in_maps = [_prep_core_inputs(c, tgt, src, Wq, Wk, Wv, Wo)
               for c in range(NCORES)]
    res = run_bass_kernel_spmd(nc, in_maps, core_ids=list(range(NCORES)))

    out = np.empty((B, N, C), dtype=np.float32)
    for c in range(NCORES):
        b, qoff = c // 4, NQ * (c % 4)
        out[b, qoff:qoff + NQ, :] = res.results[c]["yT"].T
    if _want_results:
        return out, res
    return out

